# revision 37
# baseline (speedup 1.0000x reference)
"""Self-attention kernel for Trainium2, 8 NeuronCores, data-parallel over batch.

Reference computation (per batch sample, N=H*W=4096, C=64, Ck=8):
    f = x @ Wf + bf            [N, 8]
    g = x @ Wg + bg            [N, 8]
    h = x @ Wh + bh            [N, 64]
    s = f @ g^T                [N, N]
    attn = softmax(s, axis=-1)
    o = gamma * (attn @ h) + x

Kernel strategy (one sample per core):
  - Scores computed TRANSPOSED: sT[m, n] with m (the softmax-reduction index)
    on partitions, via K=9 bf16 matmuls packed two-at-a-time into 32-row
    PE tile_position row groups.  No max subtraction (scores are O(1)); the
    softmax denominator comes free from an augmented column in h.
  - exp split across ScalarE (true exp via activation affine) and VectorE
    (fp8e4m3 Schraudolph bit-trick: i8 = max(s'/16, 0) bitcast to e4m3
    = exp(s)/8), alternating whole [128, 1024] chunks.  Scores carry a
    C1=128*log2(e) scale and +504 offset folded into the weights.
  - ctx^T = [128*gamma*h | 128]^T @ exp accumulated in PSUM over m with
    fp8 DoubleRow matmuls (two m-tiles per instruction); row 64 gives
    128*sum(exp), whose reciprocal directly yields gamma*ctx.
  - Epilogue: DMA-transpose ctxT back to [n, c] layout, per-partition
    reciprocal, scale on DVE, residual add on GpSimd, DMA out.

NOTE on attempted optimizations (profiled on HW, all reverted): packing the
K=9 score matmuls 4-at-a-time into the 32-row PE tile_position row groups
gives ~4x score-matmul concurrency when the PE is warm, BUT row-group and
DoubleRow matmuls are invisible to the PE HAM clock gate's activity monitor,
so the PE re-throttles to 1.2 GHz (and, once cold with only invisible work
queued, never re-warms: SHORT-window "busy" counts only full-array non-DR
matmuls).  The throttled clock more than cancels the packing win (best
packed variant: 164us vs 152us for this layout, which keeps the PE ~95%
busy with HAM-visible full-K matmuls).
"""

import numpy as np
import ml_dtypes

import concourse.bass as bass
import concourse.mybir as mybir
import concourse.tile as tile
from concourse.bass import ts, ds
from concourse.bass_utils import run_bass_kernel_spmd
from concourse.masks import make_identity

BF16 = mybir.dt.bfloat16
FP8 = mybir.dt.float8e4
F32 = mybir.dt.float32

N = 4096          # H*W per sample
C = 64            # channels
CK = 8            # f/g projection dim
P = 128           # partitions
NT = N // P       # 32 n/m tiles
HALF = N // 2     # 2048
HT = HALF // P    # 16 tiles per half
C1 = 128.0 * np.log2(np.e)   # score pre-scale (f side), undone by ACT affine
SCORE_OFF = 504.0            # additive score offset (exact in bf16):
                             # s'/16 = 8*log2(e)*s + 31.5, the e4m3 bit value
                             # of exp(s)/8 with the Schraudolph shift; clamped
                             # at 0 on the DVE.  ACT computes exp(s - ln 8).
                             # The /8 keeps exp in e4m3 range; softmax ratios
                             # are unaffected.

def _np_bf16(a):
    return np.ascontiguousarray(a.astype(np.float32).astype(ml_dtypes.bfloat16))


def prepare_weights(Wf, bf, Wg, bg, Wh, bh, gamma):
    """Host-side weight folding. Returns dict of bf16 arrays (dram params)."""
    Wf = np.asarray(Wf, np.float32)
    Wg = np.asarray(Wg, np.float32)
    Wh = np.asarray(Wh, np.float32)
    bf = np.asarray(bf, np.float32)
    bg = np.asarray(bg, np.float32)
    bh = np.asarray(bh, np.float32)
    gamma = float(np.asarray(gamma, np.float32))

    # f-side, scaled by C1, bias as row 64; replicated across the 4 32-row
    # bands so row-group-packed score matmuls can read from any band.
    # Column 8 (paired with g-side column 8 == 1) adds SCORE_OFF to every
    # score so the DVE fp8 bit-trick can clamp at 0 instead of going
    # negative: raw scores' = C1*s + SCORE_OFF.
    # single band: with full-K score matmuls the g-side replicas must hit
    # zero rows of fS outside band 0
    wf_aug = np.zeros((128, 128), np.float32)
    wf_aug[:C, :CK] = C1 * Wf
    wf_aug[C, :CK] = C1 * bf
    wf_aug[C, CK] = SCORE_OFF

    # g-side, same replication, column 8 = ones row
    wg_aug = np.zeros((128, 128), np.float32)
    for b in range(4):
        wg_aug[:C, 32 * b: 32 * b + CK] = Wg
        wg_aug[C, 32 * b: 32 * b + CK] = bg
        wg_aug[C, 32 * b + CK] = 1.0

    # h-side scaled by 128*gamma (keeps fp8 h out of subnormals) and a
    # 128-valued denominator column at 64; the epilogue's reciprocal of
    # 128*sum(exp) then yields gamma*ctx directly: [128, 128]
    wh_aug = np.zeros((128, 128), np.float32)
    wh_aug[:C, :C] = 128.0 * gamma * Wh
    wh_aug[C, :C] = 128.0 * gamma * bh
    wh_aug[C, C] = 128.0

    return {
        "wf": _np_bf16(wf_aug),
        "wg": _np_bf16(wg_aug),
        "wh": _np_bf16(wh_aug),
    }


def _spill_excess_waits(nc, limit=1):
    """Walrus rejects HW-queue instructions carrying more than a couple of
    semaphore waits.  Move excess waits onto standalone EventSemaphore
    instructions inserted just before the offender on the same engine
    (cumulative sem-ge waits split across instructions are equivalent)."""
    n_spill = 0
    for bb in nc.main_func.blocks:
        rebuilt = []
        changed = False
        for ins in bb.instructions:
            si = ins.sync_info
            if si is not None and len(si.on_wait) > limit:
                waits = list(si.on_wait)
                for w in waits[limit:]:
                    ev = mybir.InstEventSemaphore(
                        name=f"wspill-{n_spill}", ins=[], outs=[])
                    ev.engine = ins.engine
                    ev.sync_info = mybir.SyncInfo(on_wait=[w], on_update=[])
                    rebuilt.append(ev)
                    n_spill += 1
                ins.sync_info = mybir.SyncInfo(
                    on_wait=waits[:limit], on_update=list(si.on_update))
                changed = True
            rebuilt.append(ins)
        if changed:
            bb.instructions = rebuilt
    return n_spill


def _dedup_ldweights(nc):
    """Drop an InstLdweights whose weight AP/mode is identical to the
    immediately preceding LDW on the PE queue (score-chunk and DoubleRow
    pairs reuse the same stationary operand).  Only sync-free LDWs are
    dropped so no semaphore edges are lost."""
    n_drop = 0
    for bb in nc.main_func.blocks:
        rebuilt = []
        last_key = None
        changed = False
        for ins in bb.instructions:
            tname = type(ins).__name__
            if tname == "InstLdweights":
                si = ins.sync_info
                clean = si is None or (not si.on_wait and not si.on_update)
                key = (str(ins.ins[0]), str(getattr(ins, "perf_mode", None)),
                       str(getattr(ins, "tile_position", None)),
                       str(getattr(ins, "is_transpose", None)))
                if clean and key == last_key:
                    n_drop += 1
                    changed = True
                    continue
                last_key = key
            elif tname == "InstMatmult":
                pass  # matmul leaves the stationary operand in place
            elif ins.engine == mybir.EngineType.PE:
                last_key = None
            rebuilt.append(ins)
        if changed:
            bb.instructions = rebuilt
    return n_drop


def build_bass(repeat=1, spill=True):
    """Build the per-core Bass graph (SPMD: same graph on all 8 cores).
    repeat > 1 duplicates the whole body for timing calibration."""
    nc = bass.Bass()

    x_d = nc.declare_dram_parameter("x", [P, NT * C], F32, isOutput=False)
    wf_d = nc.declare_dram_parameter("wf", [128, 128], BF16, isOutput=False)
    wg_d = nc.declare_dram_parameter("wg", [128, 128], BF16, isOutput=False)
    wh_d = nc.declare_dram_parameter("wh", [128, 128], BF16, isOutput=False)
    xt_d = nc.declare_dram_parameter("xta", [65, N], BF16, isOutput=False)
    out_d = nc.declare_dram_parameter("out", [N, C], F32, isOutput=True)

    with tile.TileContext(nc) as tc:
        for _ in range(repeat):
            _build_body(nc, tc, x_d, wf_d, wg_d, wh_d, xt_d, out_d)
    _dedup_ldweights(nc)
    if spill:
        _spill_excess_waits(nc)
    return nc


def _build_body(nc, tc, x_d, wf_d, wg_d, wh_d, xt_d, out_d):
    from contextlib import ExitStack

    with ExitStack() as ctx:
        consts = ctx.enter_context(tc.tile_pool(name="consts", bufs=1))
        sbuf = ctx.enter_context(tc.tile_pool(name="sbuf", bufs=1))
        exp_pool = ctx.enter_context(tc.tile_pool(name="expp", bufs=6))
        work = ctx.enter_context(tc.tile_pool(name="work", bufs=6))

        # ---- load x (host pre-tiled to [p, t*c] f32): contiguous DMAs ----
        # sync (HWDGE) + gpsimd (SWDGE) queues only: keeping ALL DMA traffic
        # off the scalar queue leaves ACT purely for exp (its queue time
        # gates the ctx matmuls via the e tiles)
        x_sb = consts.tile([P, NT, C], F32)
        x3 = x_d.rearrange("p (t c) -> p t c", c=C)
        dma_engines = [nc.sync, nc.gpsimd]
        for d in range(4):
            dma_engines[d % 2].dma_start(x_sb[:, ds(4 * d, 4), :],
                                         x3[:, ds(4 * d, 4), :])
        # ---- constants (small, after x on the queues) ----
        wf_sb = consts.tile([128, 128], BF16)
        wg_sb = consts.tile([128, 128], BF16)
        wh_sb = consts.tile([128, 128], BF16)
        nc.sync.dma_start(wf_sb[:], wf_d[:])
        nc.gpsimd.dma_start(wg_sb[:], wg_d[:])
        nc.sync.dma_start(wh_sb[:], wh_d[:])


        # identity for the final-quarter PE transposes (PE is idle then)
        id_sb = consts.tile([128, 128], BF16)
        make_identity(nc, id_sb[:])

        # --- head warmup: engines are otherwise idle for ~18us of NEFF
        # startup + input DMA.  Pull the ScalarE exp table load (~2.7us)
        # and the PE HAM un-throttle (~3.4us of sustained activity) into
        # that window using zeroed scratch.
        warm = consts.tile([128, 512], BF16)
        nc.vector.memset(warm[:], 0.0)
        wtmp = consts.tile([128, 8], BF16)
        nc.scalar.activation(wtmp[:], warm[:, :8],
                             mybir.ActivationFunctionType.Exp,
                             bias=0.0, scale=1.0)
        with tc.tile_pool(name="warm_ps", bufs=1, space="PSUM") as warm_ps:
            wp = warm_ps.tile([128, 512], F32)
            for _ in range(20):
                nc.tensor.matmul(wp[:], warm[:, :128], warm[:],
                                 start=True, stop=True)

        # ACT exp bias: exp(s'/C1 + bias) = exp(s - ln 8)
        ebias = consts.tile([P, 1], F32)
        nc.vector.memset(ebias[:], float(-SCORE_OFF / C1 - np.log(8.0)))

        # ---- xT_aug [128, N] bf16: rows 0..64 host-built [x^T ; ones],
        # rows 65..127 zeroed on device ----
        xt_sb = consts.tile([128, N], BF16)
        nc.vector.memset(xt_sb[C:, :], 0.0)
        for d in range(2):
            dma_engines[d % 2].dma_start(xt_sb[:65, ds(d * HALF, HALF)],
                                         xt_d[:, ds(d * HALF, HALF)])

        with tc.tile_pool(name="pro_ps", bufs=2, space="PSUM") as pro_ps:
            # f/g projections (f scaled by C1), band-replicated
            f_sb = consts.tile([128, N], BF16)
            g_sb = consts.tile([128, N], BF16)
            for chunk in range(N // 512):
                pf = pro_ps.tile([128, 512], F32, tag="fg")
                nc.tensor.matmul(pf[:], wf_sb[:, :], xt_sb[:, ts(chunk, 512)],
                                 start=True, stop=True)
                nc.any.tensor_copy(f_sb[:, ts(chunk, 512)], pf[:])
            for chunk in range(N // 512):
                pg = pro_ps.tile([128, 512], F32, tag="fg")
                nc.tensor.matmul(pg[:], wg_sb[:, :], xt_sb[:, ts(chunk, 512)],
                                 start=True, stop=True)
                nc.any.tensor_copy(g_sb[:, ts(chunk, 512)], pg[:])

            # h_aug tiles in fp8: h_sb[:, m, :] = [128*gamma*h | 128 | 0pad];
            # adjacent m-tiles form the [128, 2, 128] DoubleRow weight pairs
            h_sb = consts.tile([P, NT, 128], FP8)
            for grp in range(NT // 4):
                ph = pro_ps.tile([128, 512], F32, tag="h")
                for j in range(4):
                    m = 4 * grp + j
                    nc.tensor.matmul(ph[:, ts(j, P)], xt_sb[:, ts(m, P)],
                                     wh_sb[:], start=True, stop=True)
                nc.any.tensor_copy(h_sb[:, ds(4 * grp, 4), :], ph[:])

        # x tiles 16..31 (quarters 2-3 residuals, needed ~90us in): behind
        # the compute-critical loads on each queue
        for d in range(4, 8):
            dma_engines[d % 2].dma_start(x_sb[:, ds(4 * d, 4), :],
                                         x3[:, ds(4 * d, 4), :])

        # ---- main: scores -> exp -> ctxT accumulate; epilogue, per n-quarter ----
        QW = 1024                      # quarter width
        NQ = N // QW                   # 4
        QT = QW // P                   # n-tiles per quarter
        with tc.tile_pool(name="ps_s", bufs=3, space="PSUM") as ps_s, \
             tc.tile_pool(name="ps_ctx", bufs=1, space="PSUM") as ps_ctx:
            # exp engine assignment: ACT chunk ~997ns vs DVE ~1192ns (+ DVE's
            # prologue/epilogue load) -> give ACT ~81 of 128 chunks, spread
            # evenly (Bresenham).
            N_CHUNKS = NQ * NT
            DVE_SHARE = 58
            use_dve = [((i * DVE_SHARE) % N_CHUNKS) < DVE_SHARE
                       for i in range(N_CHUNKS)]

            for q in range(NQ):
                ctx_ps = ps_ctx.tile([128, QW], F32, tag="ctx")
                for mg in range(NT // 2):
                    # two m-tiles' score matmuls (full-K: the zero rows of
                    # fS make the band replicas in gS harmless)
                    sp = [ps_s.tile([128, QW], F32, tag="s", name=f"s{b}")
                          for b in range(2)]
                    for b in range(2):
                        m = 2 * mg + b
                        for j in range(QW // 512):
                            nc.tensor.matmul(
                                sp[b][:, ts(j, 512)],
                                g_sb[:, ts(m, P)],
                                f_sb[:, ds(q * QW + j * 512, 512)],
                                start=True, stop=True)
                    e_pair = exp_pool.tile([128, 2, QW], FP8, tag="e")
                    for b in range(2):
                        m = 2 * mg + b
                        if use_dve[q * NT + m]:
                            _dve_exp(nc, work, e_pair, b, sp[b], QW)
                        else:
                            nc.scalar.activation(
                                e_pair[:, b, :], sp[b][:],
                                mybir.ActivationFunctionType.Exp,
                                bias=ebias[:], scale=float(1.0 / C1))
                    for j in range(QW // 512):
                        nc.tensor.matmul(
                            ctx_ps[:, ts(j, 512)],
                            h_sb[:, ds(2 * mg, 2), :],
                            e_pair[:, :, ts(j, 512)],
                            perf_mode=mybir.MatmulPerfMode.DoubleRow,
                            start=(mg == 0), stop=(mg == NT // 2 - 1))

                # epilogue for this quarter: DMA-transpose ctxT back to [n, c];
                # copy in halves so transposes start before the full copy, and
                # spread transposes/stores across engine DMA queues
                ctxt_sb = work.tile([128, QW], BF16, tag="ctxt")
                nc.any.tensor_copy(ctxt_sb[:, :QW // 2], ctx_ps[:, :QW // 2])
                nc.any.tensor_copy(ctxt_sb[:, QW // 2:], ctx_ps[:, QW // 2:])
                last_q = q == NQ - 1
                if last_q:
                    # nothing left for the PE: transpose on it instead of the
                    # DMA xbar so the tail isn't queue-serialized
                    tr_ps = ps_s.tile([128, QW], BF16, tag="s", name="trps")
                    for t in range(QT):
                        nc.tensor.transpose(tr_ps[:, ts(t, P)],
                                            ctxt_sb[:, ts(t, P)], id_sb[:])
                else:
                    o_tr = work.tile([128, QT, P], BF16, tag="otr")
                    for t in range(QT):
                        nc.sync.dma_start_transpose(
                            o_tr[:, t, :], ctxt_sb[:, ts(t, P)])
                for t in range(QT):
                    blk = tr_ps[:, ts(t, P)] if last_q else o_tr[:, t, :]
                    rden = work.tile([P, 1], F32, tag="rden")
                    nc.vector.reciprocal(rden[:], blk[:, C: C + 1])
                    tmp = work.tile([P, C], F32, tag="tmp")
                    nc.vector.tensor_scalar_mul(tmp[:], blk[:, :C], rden[:])
                    osb = work.tile([P, C], F32, tag="osb")
                    nc.gpsimd.tensor_add(osb[:], tmp[:],
                                         x_sb[:, q * QT + t, :])
                    dma_engines[(t + 1) % 2].dma_start(
                        out_d[ds((q * QT + t) * P, P), :], osb[:])


def _dve_exp(nc, work, e_pair, b, s_ps, ncols):
    """fp8e4m3 bit-trick exp on the DVE: i8 = round(max(s'/16, 0))
    reinterpreted as e4m3 ~= exp(s)/8.  s' = C1*s + SCORE_OFF (from the
    weights), so s'/16 = 8*log2(e)*s + 31.5 -- the e4m3 bit pattern of
    exp(s)/8 with the Schraudolph shift; ultra-negative scores clamp to +0."""
    i8_view = e_pair.bitcast(mybir.dt.int8)
    nc.vector.tensor_scalar(i8_view[:, b, :ncols], s_ps[:, :ncols],
                            1.0 / 16.0, 0.0,
                            mybir.AluOpType.mult, mybir.AluOpType.max)


_CACHE = {}


def _get_nc():
    if "nc" not in _CACHE:
        _CACHE["nc"] = build_bass()
    return _CACHE["nc"]


def kernel(x, Wf, bf, Wg, bg, Wh, bh, gamma):
    x = np.asarray(x, np.float32)
    B = x.shape[0]
    assert x.shape == (B, 64, 64, 64) and B == 8

    w = prepare_weights(Wf, bf, Wg, bg, Wh, bh, gamma)
    nc = _get_nc()
    xt = x.reshape(B, NT, P, C).transpose(0, 2, 1, 3).reshape(B, P, NT * C)
    xta = np.ones((B, 65, N), np.float32)
    xta[:, :C, :] = x.reshape(B, N, C).transpose(0, 2, 1)
    xta = xta.astype(ml_dtypes.bfloat16)
    in_maps = [{"x": np.ascontiguousarray(xt[i]),
                "xta": np.ascontiguousarray(xta[i]), **w} for i in range(B)]
    res = run_bass_kernel_spmd(nc, in_maps, core_ids=list(range(8)))
    out = np.stack([np.asarray(res.results[i]["out"]).reshape(64, 64, 64)
                    for i in range(B)])
    return out.astype(np.float32)


# revision 38
# speedup vs baseline: 1.1155x; 1.1155x over previous
"""Self-attention kernel for Trainium2, 8 NeuronCores, data-parallel over batch.

Reference computation (per batch sample, N=H*W=4096, C=64, Ck=8):
    f = x @ Wf + bf            [N, 8]
    g = x @ Wg + bg            [N, 8]
    h = x @ Wh + bh            [N, 64]
    s = f @ g^T                [N, N]
    attn = softmax(s, axis=-1)
    o = gamma * (attn @ h) + x

Kernel strategy (one sample per core):
  - Scores computed TRANSPOSED: sT[m, n] with m (the softmax-reduction index)
    on partitions, via K=9 bf16 matmuls packed two-at-a-time into 32-row
    PE tile_position row groups.  No max subtraction (scores are O(1)); the
    softmax denominator comes free from an augmented column in h.
  - exp split across ScalarE (true exp via activation affine) and VectorE
    (fp8e4m3 Schraudolph bit-trick: i8 = max(s'/16, 0) bitcast to e4m3
    = exp(s)/8), alternating whole [128, 1024] chunks.  Scores carry a
    C1=128*log2(e) scale and +504 offset folded into the weights.
  - ctx^T = [128*gamma*h | 128]^T @ exp accumulated in PSUM over m with
    fp8 DoubleRow matmuls (two m-tiles per instruction); row 64 gives
    128*sum(exp), whose reciprocal directly yields gamma*ctx.
  - Epilogue: DMA-transpose ctxT back to [n, c] layout, per-partition
    reciprocal, scale on DVE, residual add on GpSimd, DMA out.

NOTE on attempted optimizations (profiled on HW, all reverted): packing the
K=9 score matmuls 4-at-a-time into the 32-row PE tile_position row groups
gives ~4x score-matmul concurrency when the PE is warm, BUT row-group and
DoubleRow matmuls are invisible to the PE HAM clock gate's activity monitor,
so the PE re-throttles to 1.2 GHz (and, once cold with only invisible work
queued, never re-warms: SHORT-window "busy" counts only full-array non-DR
matmuls).  The throttled clock more than cancels the packing win (best
packed variant: 164us vs 152us for this layout, which keeps the PE ~95%
busy with HAM-visible full-K matmuls).
"""

import numpy as np
import ml_dtypes

import concourse.bass as bass
import concourse.mybir as mybir
import concourse.tile as tile
from concourse.bass import ts, ds
from concourse.bass_utils import run_bass_kernel_spmd
from concourse.masks import make_identity

BF16 = mybir.dt.bfloat16
FP8 = mybir.dt.float8e4
F32 = mybir.dt.float32

N = 4096          # H*W per sample
C = 64            # channels
CK = 8            # f/g projection dim
P = 128           # partitions
NT = N // P       # 32 n/m tiles
HALF = N // 2     # 2048
HT = HALF // P    # 16 tiles per half
C1 = 128.0 * np.log2(np.e)   # score pre-scale (f side), undone by ACT affine
SCORE_OFF = 504.0            # additive score offset (exact in bf16):
                             # s'/16 = 8*log2(e)*s + 31.5, the e4m3 bit value
                             # of exp(s)/8 with the Schraudolph shift; clamped
                             # at 0 on the DVE.  ACT computes exp(s - ln 8).
                             # The /8 keeps exp in e4m3 range; softmax ratios
                             # are unaffected.

def _np_bf16(a):
    return np.ascontiguousarray(a.astype(np.float32).astype(ml_dtypes.bfloat16))


def prepare_weights(Wf, bf, Wg, bg, Wh, bh, gamma):
    """Host-side weight folding. Returns dict of bf16 arrays (dram params)."""
    Wf = np.asarray(Wf, np.float32)
    Wg = np.asarray(Wg, np.float32)
    Wh = np.asarray(Wh, np.float32)
    bf = np.asarray(bf, np.float32)
    bg = np.asarray(bg, np.float32)
    bh = np.asarray(bh, np.float32)
    gamma = float(np.asarray(gamma, np.float32))

    # f-side, scaled by C1, bias as row 64; replicated across the 4 32-row
    # bands so row-group-packed score matmuls can read from any band.
    # Column 8 (paired with g-side column 8 == 1) adds SCORE_OFF to every
    # score so the DVE fp8 bit-trick can clamp at 0 instead of going
    # negative: raw scores' = C1*s + SCORE_OFF.
    # single band: with full-K score matmuls the g-side replicas must hit
    # zero rows of fS outside band 0
    wf_aug = np.zeros((128, 128), np.float32)
    wf_aug[:C, :CK] = C1 * Wf
    wf_aug[C, :CK] = C1 * bf
    wf_aug[C, CK] = SCORE_OFF

    # g-side, same replication, column 8 = ones row
    wg_aug = np.zeros((128, 128), np.float32)
    for b in range(4):
        wg_aug[:C, 32 * b: 32 * b + CK] = Wg
        wg_aug[C, 32 * b: 32 * b + CK] = bg
        wg_aug[C, 32 * b + CK] = 1.0

    # h-side scaled by 128*gamma (keeps fp8 h out of subnormals) and a
    # 128-valued denominator column at 64; the epilogue's reciprocal of
    # 128*sum(exp) then yields gamma*ctx directly: [128, 128]
    wh_aug = np.zeros((128, 128), np.float32)
    wh_aug[:C, :C] = 128.0 * gamma * Wh
    wh_aug[C, :C] = 128.0 * gamma * bh
    wh_aug[C, C] = 128.0

    return {
        "wf": _np_bf16(wf_aug),
        "wg": _np_bf16(wg_aug),
        "wh": _np_bf16(wh_aug),
    }


def _spill_excess_waits(nc, limit=1):
    """Walrus rejects HW-queue instructions carrying more than a couple of
    semaphore waits.  Move excess waits onto standalone EventSemaphore
    instructions inserted just before the offender on the same engine
    (cumulative sem-ge waits split across instructions are equivalent)."""
    n_spill = 0
    for bb in nc.main_func.blocks:
        rebuilt = []
        changed = False
        for ins in bb.instructions:
            si = ins.sync_info
            if si is not None and len(si.on_wait) > limit:
                waits = list(si.on_wait)
                for w in waits[limit:]:
                    ev = mybir.InstEventSemaphore(
                        name=f"wspill-{n_spill}", ins=[], outs=[])
                    ev.engine = ins.engine
                    ev.sync_info = mybir.SyncInfo(on_wait=[w], on_update=[])
                    rebuilt.append(ev)
                    n_spill += 1
                ins.sync_info = mybir.SyncInfo(
                    on_wait=waits[:limit], on_update=list(si.on_update))
                changed = True
            rebuilt.append(ins)
        if changed:
            bb.instructions = rebuilt
    return n_spill


def _dedup_ldweights(nc):
    """Drop an InstLdweights whose weight AP/mode is identical to the
    immediately preceding LDW on the PE queue (score-chunk and DoubleRow
    pairs reuse the same stationary operand).  Only sync-free LDWs are
    dropped so no semaphore edges are lost."""
    n_drop = 0
    for bb in nc.main_func.blocks:
        rebuilt = []
        last_key = None
        changed = False
        for ins in bb.instructions:
            tname = type(ins).__name__
            if tname == "InstLdweights":
                si = ins.sync_info
                clean = si is None or (not si.on_wait and not si.on_update)
                key = (str(ins.ins[0]), str(getattr(ins, "perf_mode", None)),
                       str(getattr(ins, "tile_position", None)),
                       str(getattr(ins, "is_transpose", None)))
                if clean and key == last_key:
                    n_drop += 1
                    changed = True
                    continue
                last_key = key
            elif tname == "InstMatmult":
                pass  # matmul leaves the stationary operand in place
            elif ins.engine == mybir.EngineType.PE:
                last_key = None
            rebuilt.append(ins)
        if changed:
            bb.instructions = rebuilt
    return n_drop


def build_bass(repeat=1, spill=True):
    """Build the per-core Bass graph (SPMD: same graph on all 8 cores).
    repeat > 1 duplicates the whole body for timing calibration."""
    nc = bass.Bass()

    x_d = nc.declare_dram_parameter("x", [P, NT * C], F32, isOutput=False)
    wf_d = nc.declare_dram_parameter("wf", [128, 128], BF16, isOutput=False)
    wg_d = nc.declare_dram_parameter("wg", [128, 128], BF16, isOutput=False)
    wh_d = nc.declare_dram_parameter("wh", [128, 128], BF16, isOutput=False)
    xt_d = nc.declare_dram_parameter("xta", [65, N], BF16, isOutput=False)
    out_d = nc.declare_dram_parameter("out", [N, C], F32, isOutput=True)

    with tile.TileContext(nc) as tc:
        for _ in range(repeat):
            _build_body(nc, tc, x_d, wf_d, wg_d, wh_d, xt_d, out_d)
    _dedup_ldweights(nc)
    if spill:
        _spill_excess_waits(nc)
    return nc


def _build_body(nc, tc, x_d, wf_d, wg_d, wh_d, xt_d, out_d):
    from contextlib import ExitStack

    with ExitStack() as ctx:
        consts = ctx.enter_context(tc.tile_pool(name="consts", bufs=1))
        sbuf = ctx.enter_context(tc.tile_pool(name="sbuf", bufs=1))
        exp_pool = ctx.enter_context(tc.tile_pool(name="expp", bufs=6))
        work = ctx.enter_context(tc.tile_pool(name="work", bufs=6))

        # ---- load x (host pre-tiled to [p, t*c] f32): contiguous DMAs ----
        x_sb = consts.tile([P, NT, C], F32)
        x3 = x_d.rearrange("p (t c) -> p t c", c=C)
        dma_engines = [nc.sync, nc.gpsimd, nc.scalar]
        for d in range(4):
            dma_engines[d % 3].dma_start(x_sb[:, ds(4 * d, 4), :],
                                         x3[:, ds(4 * d, 4), :])
        # ---- constants (small, after x on the queues) ----
        wf_sb = consts.tile([128, 128], BF16)
        wg_sb = consts.tile([128, 128], BF16)
        wh_sb = consts.tile([128, 128], BF16)
        nc.sync.dma_start(wf_sb[:], wf_d[:])
        nc.gpsimd.dma_start(wg_sb[:], wg_d[:])
        nc.scalar.dma_start(wh_sb[:], wh_d[:])


        # identity for the final-quarter PE transposes (PE is idle then)
        id_sb = consts.tile([128, 128], BF16)
        make_identity(nc, id_sb[:])

        # --- head warmup: engines are otherwise idle for ~18us of NEFF
        # startup + input DMA.  Pull the ScalarE exp table load (~2.7us)
        # and the PE HAM un-throttle (~3.4us of sustained activity) into
        # that window using zeroed scratch.
        warm = consts.tile([128, 512], BF16)
        nc.vector.memset(warm[:], 0.0)
        wtmp = consts.tile([128, 8], BF16)
        nc.scalar.activation(wtmp[:], warm[:, :8],
                             mybir.ActivationFunctionType.Exp,
                             bias=0.0, scale=1.0)
        with tc.tile_pool(name="warm_ps", bufs=1, space="PSUM") as warm_ps:
            wp = warm_ps.tile([128, 512], F32)
            for _ in range(20):
                nc.tensor.matmul(wp[:], warm[:, :128], warm[:],
                                 start=True, stop=True)

        # ACT exp bias: exp(s'/C1 + bias) = exp(s - ln 8)
        ebias = consts.tile([P, 1], F32)
        nc.vector.memset(ebias[:], float(-SCORE_OFF / C1 - np.log(8.0)))

        # ---- xT_aug [128, N] bf16: rows 0..64 host-built [x^T ; ones],
        # rows 65..127 zeroed on device ----
        xt_sb = consts.tile([128, N], BF16)
        nc.vector.memset(xt_sb[C:, :], 0.0)
        for d in range(2):
            dma_engines[d % 3].dma_start(xt_sb[:65, ds(d * HALF, HALF)],
                                         xt_d[:, ds(d * HALF, HALF)])

        with tc.tile_pool(name="pro_ps", bufs=2, space="PSUM") as pro_ps:
            # f/g projections (f scaled by C1), band-replicated
            f_sb = consts.tile([128, N], BF16)
            g_sb = consts.tile([128, N], BF16)
            for chunk in range(N // 512):
                pf = pro_ps.tile([128, 512], F32, tag="fg")
                nc.tensor.matmul(pf[:], wf_sb[:, :], xt_sb[:, ts(chunk, 512)],
                                 start=True, stop=True)
                nc.any.tensor_copy(f_sb[:, ts(chunk, 512)], pf[:])
            for chunk in range(N // 512):
                pg = pro_ps.tile([128, 512], F32, tag="fg")
                nc.tensor.matmul(pg[:], wg_sb[:, :], xt_sb[:, ts(chunk, 512)],
                                 start=True, stop=True)
                nc.any.tensor_copy(g_sb[:, ts(chunk, 512)], pg[:])

            # h_aug tiles in fp8: h_sb[:, m, :] = [128*gamma*h | 128 | 0pad];
            # adjacent m-tiles form the [128, 2, 128] DoubleRow weight pairs
            h_sb = consts.tile([P, NT, 128], FP8)
            for grp in range(NT // 4):
                ph = pro_ps.tile([128, 512], F32, tag="h")
                for j in range(4):
                    m = 4 * grp + j
                    nc.tensor.matmul(ph[:, ts(j, P)], xt_sb[:, ts(m, P)],
                                     wh_sb[:], start=True, stop=True)
                nc.any.tensor_copy(h_sb[:, ds(4 * grp, 4), :], ph[:])

        # x tiles 16..31 (quarters 2-3 residuals, needed ~90us in): behind
        # the compute-critical loads on each queue
        for d in range(4, 8):
            dma_engines[d % 3].dma_start(x_sb[:, ds(4 * d, 4), :],
                                         x3[:, ds(4 * d, 4), :])

        # ---- main: scores -> exp -> ctxT accumulate; epilogue, per n-quarter ----
        QW = 1024                      # quarter width
        NQ = N // QW                   # 4
        QT = QW // P                   # n-tiles per quarter
        with tc.tile_pool(name="ps_s", bufs=3, space="PSUM") as ps_s, \
             tc.tile_pool(name="ps_ctx", bufs=1, space="PSUM") as ps_ctx:
            # exp engine assignment: ACT chunk ~997ns vs DVE ~1192ns (+ DVE's
            # prologue/epilogue load) -> give ACT ~81 of 128 chunks, spread
            # evenly (Bresenham).
            N_CHUNKS = NQ * NT
            DVE_SHARE = 58
            use_dve = [((i * DVE_SHARE) % N_CHUNKS) < DVE_SHARE
                       for i in range(N_CHUNKS)]

            for q in range(NQ):
                ctx_ps = ps_ctx.tile([128, QW], F32, tag="ctx")
                for mg in range(NT // 2):
                    # two m-tiles' score matmuls (full-K: the zero rows of
                    # fS make the band replicas in gS harmless)
                    sp = [ps_s.tile([128, QW], F32, tag="s", name=f"s{b}")
                          for b in range(2)]
                    for b in range(2):
                        m = 2 * mg + b
                        for j in range(QW // 512):
                            nc.tensor.matmul(
                                sp[b][:, ts(j, 512)],
                                g_sb[:, ts(m, P)],
                                f_sb[:, ds(q * QW + j * 512, 512)],
                                start=True, stop=True)
                    e_pair = exp_pool.tile([128, 2, QW], FP8, tag="e")
                    for b in range(2):
                        m = 2 * mg + b
                        if use_dve[q * NT + m]:
                            _dve_exp(nc, work, e_pair, b, sp[b], QW)
                        else:
                            nc.scalar.activation(
                                e_pair[:, b, :], sp[b][:],
                                mybir.ActivationFunctionType.Exp,
                                bias=ebias[:], scale=float(1.0 / C1))
                    for j in range(QW // 512):
                        nc.tensor.matmul(
                            ctx_ps[:, ts(j, 512)],
                            h_sb[:, ds(2 * mg, 2), :],
                            e_pair[:, :, ts(j, 512)],
                            perf_mode=mybir.MatmulPerfMode.DoubleRow,
                            start=(mg == 0), stop=(mg == NT // 2 - 1))

                # epilogue for this quarter: DMA-transpose ctxT back to [n, c];
                # copy in halves so transposes start before the full copy, and
                # spread transposes/stores across engine DMA queues
                ctxt_sb = work.tile([128, QW], BF16, tag="ctxt")
                nc.any.tensor_copy(ctxt_sb[:, :QW // 2], ctx_ps[:, :QW // 2])
                nc.any.tensor_copy(ctxt_sb[:, QW // 2:], ctx_ps[:, QW // 2:])
                last_q = q == NQ - 1
                if last_q:
                    # nothing left for the PE: transpose on it instead of the
                    # DMA xbar so the tail isn't queue-serialized
                    tr_ps = ps_s.tile([128, QW], BF16, tag="s", name="trps")
                    for t in range(QT):
                        nc.tensor.transpose(tr_ps[:, ts(t, P)],
                                            ctxt_sb[:, ts(t, P)], id_sb[:])
                else:
                    o_tr = work.tile([128, QT, P], BF16, tag="otr")
                    for t in range(QT):
                        (nc.sync if t % 2 == 0 else nc.scalar).dma_start_transpose(
                            o_tr[:, t, :], ctxt_sb[:, ts(t, P)])
                for t in range(QT):
                    blk = tr_ps[:, ts(t, P)] if last_q else o_tr[:, t, :]
                    rden = work.tile([P, 1], F32, tag="rden")
                    nc.vector.reciprocal(rden[:], blk[:, C: C + 1])
                    tmp = work.tile([P, C], F32, tag="tmp")
                    nc.vector.tensor_scalar_mul(tmp[:], blk[:, :C], rden[:])
                    osb = work.tile([P, C], F32, tag="osb")
                    nc.gpsimd.tensor_add(osb[:], tmp[:],
                                         x_sb[:, q * QT + t, :])
                    dma_engines[(t + 1) % 3].dma_start(
                        out_d[ds((q * QT + t) * P, P), :], osb[:])


def _dve_exp(nc, work, e_pair, b, s_ps, ncols):
    """fp8e4m3 bit-trick exp on the DVE: i8 = round(max(s'/16, 0))
    reinterpreted as e4m3 ~= exp(s)/8.  s' = C1*s + SCORE_OFF (from the
    weights), so s'/16 = 8*log2(e)*s + 31.5 -- the e4m3 bit pattern of
    exp(s)/8 with the Schraudolph shift; ultra-negative scores clamp to +0."""
    i8_view = e_pair.bitcast(mybir.dt.int8)
    nc.vector.tensor_scalar(i8_view[:, b, :ncols], s_ps[:, :ncols],
                            1.0 / 16.0, 0.0,
                            mybir.AluOpType.mult, mybir.AluOpType.max)


_CACHE = {}


def _get_nc():
    if "nc" not in _CACHE:
        _CACHE["nc"] = build_bass()
    return _CACHE["nc"]


def kernel(x, Wf, bf, Wg, bg, Wh, bh, gamma):
    x = np.asarray(x, np.float32)
    B = x.shape[0]
    assert x.shape == (B, 64, 64, 64) and B == 8

    w = prepare_weights(Wf, bf, Wg, bg, Wh, bh, gamma)
    nc = _get_nc()
    xt = x.reshape(B, NT, P, C).transpose(0, 2, 1, 3).reshape(B, P, NT * C)
    xta = np.ones((B, 65, N), np.float32)
    xta[:, :C, :] = x.reshape(B, N, C).transpose(0, 2, 1)
    xta = xta.astype(ml_dtypes.bfloat16)
    in_maps = [{"x": np.ascontiguousarray(xt[i]),
                "xta": np.ascontiguousarray(xta[i]), **w} for i in range(B)]
    res = run_bass_kernel_spmd(nc, in_maps, core_ids=list(range(8)))
    out = np.stack([np.asarray(res.results[i]["out"]).reshape(64, 64, 64)
                    for i in range(B)])
    return out.astype(np.float32)


# revision 39
# speedup vs baseline: 1.1242x; 1.0078x over previous
"""Self-attention kernel for Trainium2, 8 NeuronCores, data-parallel over batch.

Reference computation (per batch sample, N=H*W=4096, C=64, Ck=8):
    f = x @ Wf + bf            [N, 8]
    g = x @ Wg + bg            [N, 8]
    h = x @ Wh + bh            [N, 64]
    s = f @ g^T                [N, N]
    attn = softmax(s, axis=-1)
    o = gamma * (attn @ h) + x

Kernel strategy (one sample per core):
  - Scores computed TRANSPOSED: sT[m, n] with m (the softmax-reduction index)
    on partitions, via K=9 bf16 matmuls packed two-at-a-time into 32-row
    PE tile_position row groups.  No max subtraction (scores are O(1)); the
    softmax denominator comes free from an augmented column in h.
  - exp split across ScalarE (true exp via activation affine) and VectorE
    (fp8e4m3 Schraudolph bit-trick: i8 = max(s'/16, 0) bitcast to e4m3
    = exp(s)/8), alternating whole [128, 1024] chunks.  Scores carry a
    C1=128*log2(e) scale and +504 offset folded into the weights.
  - ctx^T = [128*gamma*h | 128]^T @ exp accumulated in PSUM over m with
    fp8 DoubleRow matmuls (two m-tiles per instruction); row 64 gives
    128*sum(exp), whose reciprocal directly yields gamma*ctx.
  - Epilogue: DMA-transpose ctxT back to [n, c] layout, per-partition
    reciprocal, scale on DVE, residual add on GpSimd, DMA out.

NOTE on attempted optimizations (profiled on HW, all reverted): packing the
K=9 score matmuls 4-at-a-time into the 32-row PE tile_position row groups
gives ~4x score-matmul concurrency when the PE is warm, BUT row-group and
DoubleRow matmuls are invisible to the PE HAM clock gate's activity monitor,
so the PE re-throttles to 1.2 GHz (and, once cold with only invisible work
queued, never re-warms: SHORT-window "busy" counts only full-array non-DR
matmuls).  The throttled clock more than cancels the packing win (best
packed variant: 164us vs 152us for this layout, which keeps the PE ~95%
busy with HAM-visible full-K matmuls).
"""

import numpy as np
import ml_dtypes

import concourse.bass as bass
import concourse.mybir as mybir
import concourse.tile as tile
from concourse.bass import ts, ds
from concourse.bass_utils import run_bass_kernel_spmd
from concourse.masks import make_identity

BF16 = mybir.dt.bfloat16
FP8 = mybir.dt.float8e4
F32 = mybir.dt.float32

N = 4096          # H*W per sample
C = 64            # channels
CK = 8            # f/g projection dim
P = 128           # partitions
NT = N // P       # 32 n/m tiles
HALF = N // 2     # 2048
HT = HALF // P    # 16 tiles per half
C1 = 128.0 * np.log2(np.e)   # score pre-scale (f side), undone by ACT affine
SCORE_OFF = 504.0            # additive score offset (exact in bf16):
                             # s'/16 = 8*log2(e)*s + 31.5, the e4m3 bit value
                             # of exp(s)/8 with the Schraudolph shift; clamped
                             # at 0 on the DVE.  ACT computes exp(s - ln 8).
                             # The /8 keeps exp in e4m3 range; softmax ratios
                             # are unaffected.

def _np_bf16(a):
    return np.ascontiguousarray(a.astype(np.float32).astype(ml_dtypes.bfloat16))


def prepare_weights(Wf, bf, Wg, bg, Wh, bh, gamma):
    """Host-side weight folding. Returns dict of bf16 arrays (dram params)."""
    Wf = np.asarray(Wf, np.float32)
    Wg = np.asarray(Wg, np.float32)
    Wh = np.asarray(Wh, np.float32)
    bf = np.asarray(bf, np.float32)
    bg = np.asarray(bg, np.float32)
    bh = np.asarray(bh, np.float32)
    gamma = float(np.asarray(gamma, np.float32))

    # f-side, scaled by C1, bias as row 64; replicated across the 4 32-row
    # bands so row-group-packed score matmuls can read from any band.
    # Column 8 (paired with g-side column 8 == 1) adds SCORE_OFF to every
    # score so the DVE fp8 bit-trick can clamp at 0 instead of going
    # negative: raw scores' = C1*s + SCORE_OFF.
    # single band: with full-K score matmuls the g-side replicas must hit
    # zero rows of fS outside band 0
    wf_aug = np.zeros((128, 128), np.float32)
    wf_aug[:C, :CK] = C1 * Wf
    wf_aug[C, :CK] = C1 * bf
    wf_aug[C, CK] = SCORE_OFF

    # g-side, same replication, column 8 = ones row
    wg_aug = np.zeros((128, 128), np.float32)
    for b in range(4):
        wg_aug[:C, 32 * b: 32 * b + CK] = Wg
        wg_aug[C, 32 * b: 32 * b + CK] = bg
        wg_aug[C, 32 * b + CK] = 1.0

    # h-side scaled by 128*gamma (keeps fp8 h out of subnormals) and a
    # 128-valued denominator column at 64; the epilogue's reciprocal of
    # 128*sum(exp) then yields gamma*ctx directly: [128, 128]
    wh_aug = np.zeros((128, 128), np.float32)
    wh_aug[:C, :C] = 128.0 * gamma * Wh
    wh_aug[C, :C] = 128.0 * gamma * bh
    wh_aug[C, C] = 128.0

    return {
        "wf": _np_bf16(wf_aug),
        "wg": _np_bf16(wg_aug),
        "wh": _np_bf16(wh_aug),
    }


def _spill_excess_waits(nc, limit=1):
    """Walrus rejects HW-queue instructions carrying more than a couple of
    semaphore waits.  Move excess waits onto standalone EventSemaphore
    instructions inserted just before the offender on the same engine
    (cumulative sem-ge waits split across instructions are equivalent)."""
    n_spill = 0
    for bb in nc.main_func.blocks:
        rebuilt = []
        changed = False
        for ins in bb.instructions:
            si = ins.sync_info
            if si is not None and len(si.on_wait) > limit:
                waits = list(si.on_wait)
                for w in waits[limit:]:
                    ev = mybir.InstEventSemaphore(
                        name=f"wspill-{n_spill}", ins=[], outs=[])
                    ev.engine = ins.engine
                    ev.sync_info = mybir.SyncInfo(on_wait=[w], on_update=[])
                    rebuilt.append(ev)
                    n_spill += 1
                ins.sync_info = mybir.SyncInfo(
                    on_wait=waits[:limit], on_update=list(si.on_update))
                changed = True
            rebuilt.append(ins)
        if changed:
            bb.instructions = rebuilt
    return n_spill


def _dedup_ldweights(nc):
    """Drop an InstLdweights whose weight AP/mode is identical to the
    immediately preceding LDW on the PE queue (score-chunk and DoubleRow
    pairs reuse the same stationary operand).  Only sync-free LDWs are
    dropped so no semaphore edges are lost."""
    n_drop = 0
    for bb in nc.main_func.blocks:
        rebuilt = []
        last_key = None
        changed = False
        for ins in bb.instructions:
            tname = type(ins).__name__
            if tname == "InstLdweights":
                si = ins.sync_info
                clean = si is None or (not si.on_wait and not si.on_update)
                key = (str(ins.ins[0]), str(getattr(ins, "perf_mode", None)),
                       str(getattr(ins, "tile_position", None)),
                       str(getattr(ins, "is_transpose", None)))
                if clean and key == last_key:
                    n_drop += 1
                    changed = True
                    continue
                last_key = key
            elif tname == "InstMatmult":
                pass  # matmul leaves the stationary operand in place
            elif ins.engine == mybir.EngineType.PE:
                last_key = None
            rebuilt.append(ins)
        if changed:
            bb.instructions = rebuilt
    return n_drop


def build_bass(repeat=1, spill=True):
    """Build the per-core Bass graph (SPMD: same graph on all 8 cores).
    repeat > 1 duplicates the whole body for timing calibration."""
    nc = bass.Bass()

    x_d = nc.declare_dram_parameter("x", [P, NT * C], F32, isOutput=False)
    wf_d = nc.declare_dram_parameter("wf", [128, 128], BF16, isOutput=False)
    wg_d = nc.declare_dram_parameter("wg", [128, 128], BF16, isOutput=False)
    wh_d = nc.declare_dram_parameter("wh", [128, 128], BF16, isOutput=False)
    xt_d = nc.declare_dram_parameter("xta", [65, N], BF16, isOutput=False)
    out_d = nc.declare_dram_parameter("out", [N, C], F32, isOutput=True)

    with tile.TileContext(nc) as tc:
        for _ in range(repeat):
            _build_body(nc, tc, x_d, wf_d, wg_d, wh_d, xt_d, out_d)
    _dedup_ldweights(nc)
    if spill:
        _spill_excess_waits(nc)
    return nc


def _build_body(nc, tc, x_d, wf_d, wg_d, wh_d, xt_d, out_d):
    from contextlib import ExitStack

    with ExitStack() as ctx:
        consts = ctx.enter_context(tc.tile_pool(name="consts", bufs=1))
        sbuf = ctx.enter_context(tc.tile_pool(name="sbuf", bufs=1))
        exp_pool = ctx.enter_context(tc.tile_pool(name="expp", bufs=6))
        work = ctx.enter_context(tc.tile_pool(name="work", bufs=6))

        # ---- load x (host pre-tiled to [p, t*c] f32): contiguous DMAs ----
        x_sb = consts.tile([P, NT, C], F32)
        x3 = x_d.rearrange("p (t c) -> p t c", c=C)
        dma_engines = [nc.sync, nc.gpsimd, nc.scalar]
        for d in range(4):
            dma_engines[d % 3].dma_start(x_sb[:, ds(4 * d, 4), :],
                                         x3[:, ds(4 * d, 4), :])
        # ---- constants (small, after x on the queues) ----
        wf_sb = consts.tile([128, 128], BF16)
        wg_sb = consts.tile([128, 128], BF16)
        wh_sb = consts.tile([128, 128], BF16)
        nc.sync.dma_start(wf_sb[:], wf_d[:])
        nc.gpsimd.dma_start(wg_sb[:], wg_d[:])
        nc.scalar.dma_start(wh_sb[:], wh_d[:])


        # identity for the final-quarter PE transposes (PE is idle then)
        id_sb = consts.tile([128, 128], BF16)
        make_identity(nc, id_sb[:])

        # --- head warmup: engines are otherwise idle for ~18us of NEFF
        # startup + input DMA.  Pull the ScalarE exp table load (~2.7us)
        # and the PE HAM un-throttle (~3.4us of sustained activity) into
        # that window using zeroed scratch.
        warm = consts.tile([128, 512], BF16)
        nc.vector.memset(warm[:], 0.0)
        wtmp = consts.tile([128, 8], BF16)
        nc.scalar.activation(wtmp[:], warm[:, :8],
                             mybir.ActivationFunctionType.Exp,
                             bias=0.0, scale=1.0)
        with tc.tile_pool(name="warm_ps", bufs=1, space="PSUM") as warm_ps:
            wp = warm_ps.tile([128, 512], F32)
            for _ in range(20):
                nc.tensor.matmul(wp[:], warm[:, :128], warm[:],
                                 start=True, stop=True)

        # ACT exp bias: exp(s'/C1 + bias) = exp(s - ln 8)
        ebias = consts.tile([P, 1], F32)
        nc.vector.memset(ebias[:], float(-SCORE_OFF / C1 - np.log(8.0)))

        # ---- xT_aug [128, N] bf16: rows 0..64 host-built [x^T ; ones],
        # rows 65..127 zeroed on device ----
        xt_sb = consts.tile([128, N], BF16)
        nc.vector.memset(xt_sb[C:, :], 0.0)
        for d in range(2):
            dma_engines[d % 3].dma_start(xt_sb[:65, ds(d * HALF, HALF)],
                                         xt_d[:, ds(d * HALF, HALF)])

        with tc.tile_pool(name="pro_ps", bufs=2, space="PSUM") as pro_ps:
            # f/g projections (f scaled by C1), band-replicated
            f_sb = consts.tile([128, N], BF16)
            g_sb = consts.tile([128, N], BF16)
            for chunk in range(N // 512):
                pf = pro_ps.tile([128, 512], F32, tag="fg")
                nc.tensor.matmul(pf[:], wf_sb[:, :], xt_sb[:, ts(chunk, 512)],
                                 start=True, stop=True)
                nc.any.tensor_copy(f_sb[:, ts(chunk, 512)], pf[:])
            for chunk in range(N // 512):
                pg = pro_ps.tile([128, 512], F32, tag="fg")
                nc.tensor.matmul(pg[:], wg_sb[:, :], xt_sb[:, ts(chunk, 512)],
                                 start=True, stop=True)
                nc.any.tensor_copy(g_sb[:, ts(chunk, 512)], pg[:])

            # h_aug tiles in fp8: h_sb[:, m, :] = [128*gamma*h | 128 | 0pad];
            # adjacent m-tiles form the [128, 2, 128] DoubleRow weight pairs
            h_sb = consts.tile([P, NT, 128], FP8)
            for grp in range(NT // 4):
                ph = pro_ps.tile([128, 512], F32, tag="h")
                for j in range(4):
                    m = 4 * grp + j
                    nc.tensor.matmul(ph[:, ts(j, P)], xt_sb[:, ts(m, P)],
                                     wh_sb[:], start=True, stop=True)
                nc.any.tensor_copy(h_sb[:, ds(4 * grp, 4), :], ph[:])

        # x tiles 16..31 (quarters 2-3 residuals, needed ~90us in): behind
        # the compute-critical loads on each queue
        for d in range(4, 8):
            dma_engines[d % 3].dma_start(x_sb[:, ds(4 * d, 4), :],
                                         x3[:, ds(4 * d, 4), :])

        # ---- main: scores -> exp -> ctxT accumulate; epilogue, per n-quarter ----
        QW = 1024                      # quarter width
        NQ = N // QW                   # 4
        QT = QW // P                   # n-tiles per quarter
        with tc.tile_pool(name="ps_s", bufs=3, space="PSUM") as ps_s, \
             tc.tile_pool(name="ps_ctx", bufs=1, space="PSUM") as ps_ctx:
            # exp engine assignment: ACT chunk ~997ns vs DVE ~1192ns (+ DVE's
            # prologue/epilogue load) -> give ACT ~81 of 128 chunks, spread
            # evenly (Bresenham).
            N_CHUNKS = NQ * NT
            DVE_SHARE = 58
            use_dve = [((i * DVE_SHARE) % N_CHUNKS) < DVE_SHARE
                       for i in range(N_CHUNKS)]

            def emit_epi_head(q, ctx_ps):
                # epilogue head for quarter q: copy ctxT to SBUF bf16 (halves
                # so transposes start before the full copy) and kick off the
                # 8 DMA transposes.  Emitted DEFERRED, inside the next
                # quarter's first group, so the engine queues don't serialize
                # the quarter boundary through the PSUM->SBUF copy.
                ctxt_sb = work.tile([128, QW], BF16, tag="ctxt")
                nc.any.tensor_copy(ctxt_sb[:, :QW // 2], ctx_ps[:, :QW // 2])
                nc.any.tensor_copy(ctxt_sb[:, QW // 2:], ctx_ps[:, QW // 2:])
                o_tr = work.tile([128, QT, P], BF16, tag="otr")
                for t in range(QT):
                    (nc.sync if t % 2 == 0 else nc.scalar).dma_start_transpose(
                        o_tr[:, t, :], ctxt_sb[:, ts(t, P)])
                return o_tr

            def emit_epi_tile(q, blk, t):
                # one n-tile of quarter q's epilogue: reciprocal of the
                # denominator column, scale, residual add, store.  Spread
                # one-per-group across the next quarter so the DVE meets
                # each DMA transpose at its natural cadence instead of
                # joining on the whole serialized chain.
                rden = work.tile([P, 1], F32, tag="rden")
                nc.vector.reciprocal(rden[:], blk[:, C: C + 1])
                tmp = work.tile([P, C], F32, tag="tmp")
                nc.vector.tensor_scalar_mul(tmp[:], blk[:, :C], rden[:])
                osb = work.tile([P, C], F32, tag="osb")
                nc.gpsimd.tensor_add(osb[:], tmp[:],
                                     x_sb[:, q * QT + t, :])
                dma_engines[(t + 1) % 3].dma_start(
                    out_d[ds((q * QT + t) * P, P), :], osb[:])

            prev_ctx = None
            prev_otr = None
            for q in range(NQ):
                ctx_ps = ps_ctx.tile([128, QW], F32, tag="ctx")
                for mg in range(NT // 2):
                    # two m-tiles' score matmuls (full-K: the zero rows of
                    # fS make the band replicas in gS harmless)
                    sp = [ps_s.tile([128, QW], F32, tag="s", name=f"s{b}")
                          for b in range(2)]
                    for b in range(2):
                        m = 2 * mg + b
                        for j in range(QW // 512):
                            nc.tensor.matmul(
                                sp[b][:, ts(j, 512)],
                                g_sb[:, ts(m, P)],
                                f_sb[:, ds(q * QW + j * 512, 512)],
                                start=True, stop=True)
                    e_pair = exp_pool.tile([128, 2, QW], FP8, tag="e")
                    for b in range(2):
                        m = 2 * mg + b
                        if use_dve[q * NT + m]:
                            _dve_exp(nc, work, e_pair, b, sp[b], QW)
                        else:
                            nc.scalar.activation(
                                e_pair[:, b, :], sp[b][:],
                                mybir.ActivationFunctionType.Exp,
                                bias=ebias[:], scale=float(1.0 / C1))
                    # previous quarter's epilogue, software-pipelined into
                    # this quarter: head at mg=0 (must precede this quarter's
                    # first ctx matmuls -- ps_ctx has bufs=1, so the copies
                    # must be emitted before the bank reuse), one tile's tail
                    # per group at mg=2..9
                    if prev_ctx is not None:
                        if mg == 0:
                            prev_otr = emit_epi_head(q - 1, prev_ctx)
                        if 2 <= mg <= 9:
                            emit_epi_tile(q - 1, prev_otr[:, mg - 2, :],
                                          mg - 2)
                    for j in range(QW // 512):
                        nc.tensor.matmul(
                            ctx_ps[:, ts(j, 512)],
                            h_sb[:, ds(2 * mg, 2), :],
                            e_pair[:, :, ts(j, 512)],
                            perf_mode=mybir.MatmulPerfMode.DoubleRow,
                            start=(mg == 0), stop=(mg == NT // 2 - 1))
                prev_ctx = ctx_ps

            # tail: last quarter's epilogue.  Nothing left for the PE, so
            # transpose on it instead of the DMA xbar; the exp engines are
            # done and run the per-tile tails back-to-back.
            ctxt_sb = work.tile([128, QW], BF16, tag="ctxt")
            nc.any.tensor_copy(ctxt_sb[:, :QW // 2], prev_ctx[:, :QW // 2])
            nc.any.tensor_copy(ctxt_sb[:, QW // 2:], prev_ctx[:, QW // 2:])
            tr_ps = ps_s.tile([128, QW], BF16, tag="s", name="trps")
            for t in range(QT):
                nc.tensor.transpose(tr_ps[:, ts(t, P)],
                                    ctxt_sb[:, ts(t, P)], id_sb[:])
            for t in range(QT):
                emit_epi_tile(NQ - 1, tr_ps[:, ts(t, P)], t)


def _dve_exp(nc, work, e_pair, b, s_ps, ncols):
    """fp8e4m3 bit-trick exp on the DVE: i8 = round(max(s'/16, 0))
    reinterpreted as e4m3 ~= exp(s)/8.  s' = C1*s + SCORE_OFF (from the
    weights), so s'/16 = 8*log2(e)*s + 31.5 -- the e4m3 bit pattern of
    exp(s)/8 with the Schraudolph shift; ultra-negative scores clamp to +0."""
    i8_view = e_pair.bitcast(mybir.dt.int8)
    nc.vector.tensor_scalar(i8_view[:, b, :ncols], s_ps[:, :ncols],
                            1.0 / 16.0, 0.0,
                            mybir.AluOpType.mult, mybir.AluOpType.max)


_CACHE = {}


def _get_nc():
    if "nc" not in _CACHE:
        _CACHE["nc"] = build_bass()
    return _CACHE["nc"]


def kernel(x, Wf, bf, Wg, bg, Wh, bh, gamma):
    x = np.asarray(x, np.float32)
    B = x.shape[0]
    assert x.shape == (B, 64, 64, 64) and B == 8

    w = prepare_weights(Wf, bf, Wg, bg, Wh, bh, gamma)
    nc = _get_nc()
    xt = x.reshape(B, NT, P, C).transpose(0, 2, 1, 3).reshape(B, P, NT * C)
    xta = np.ones((B, 65, N), np.float32)
    xta[:, :C, :] = x.reshape(B, N, C).transpose(0, 2, 1)
    xta = xta.astype(ml_dtypes.bfloat16)
    in_maps = [{"x": np.ascontiguousarray(xt[i]),
                "xta": np.ascontiguousarray(xta[i]), **w} for i in range(B)]
    res = run_bass_kernel_spmd(nc, in_maps, core_ids=list(range(8)))
    out = np.stack([np.asarray(res.results[i]["out"]).reshape(64, 64, 64)
                    for i in range(B)])
    return out.astype(np.float32)


# revision 40
# speedup vs baseline: 1.1520x; 1.0247x over previous
"""Self-attention kernel for Trainium2, 8 NeuronCores, data-parallel over batch.

Reference computation (per batch sample, N=H*W=4096, C=64, Ck=8):
    f = x @ Wf + bf            [N, 8]
    g = x @ Wg + bg            [N, 8]
    h = x @ Wh + bh            [N, 64]
    s = f @ g^T                [N, N]
    attn = softmax(s, axis=-1)
    o = gamma * (attn @ h) + x

Kernel strategy (one sample per core):
  - Scores computed TRANSPOSED: sT[m, n] with m (the softmax-reduction index)
    on partitions, via K=9 bf16 matmuls packed two-at-a-time into 32-row
    PE tile_position row groups.  No max subtraction (scores are O(1)); the
    softmax denominator comes free from an augmented column in h.
  - exp split across ScalarE (true exp via activation affine) and VectorE
    (fp8e4m3 Schraudolph bit-trick: i8 = max(s'/16, 0) bitcast to e4m3
    = exp(s)/8), alternating whole [128, 1024] chunks.  Scores carry a
    C1=128*log2(e) scale and +504 offset folded into the weights.
  - ctx^T = [128*gamma*h | 128]^T @ exp accumulated in PSUM over m with
    fp8 DoubleRow matmuls (two m-tiles per instruction); row 64 gives
    128*sum(exp), whose reciprocal directly yields gamma*ctx.
  - Epilogue: DMA-transpose ctxT back to [n, c] layout, per-partition
    reciprocal, scale on DVE, residual add on GpSimd, DMA out.

NOTE on attempted optimizations (profiled on HW, all reverted): packing the
K=9 score matmuls 4-at-a-time into the 32-row PE tile_position row groups
gives ~4x score-matmul concurrency when the PE is warm, BUT row-group and
DoubleRow matmuls are invisible to the PE HAM clock gate's activity monitor,
so the PE re-throttles to 1.2 GHz (and, once cold with only invisible work
queued, never re-warms: SHORT-window "busy" counts only full-array non-DR
matmuls).  The throttled clock more than cancels the packing win (best
packed variant: 164us vs 152us for this layout, which keeps the PE ~95%
busy with HAM-visible full-K matmuls).
"""

import numpy as np
import ml_dtypes

import concourse.bass as bass
import concourse.mybir as mybir
import concourse.tile as tile
from concourse.bass import ts, ds
from concourse.bass_utils import run_bass_kernel_spmd
from concourse.masks import make_identity

BF16 = mybir.dt.bfloat16
FP8 = mybir.dt.float8e4
F32 = mybir.dt.float32

N = 4096          # H*W per sample
C = 64            # channels
CK = 8            # f/g projection dim
P = 128           # partitions
NT = N // P       # 32 n/m tiles
HALF = N // 2     # 2048
HT = HALF // P    # 16 tiles per half
C1 = 128.0 * np.log2(np.e)   # score pre-scale (f side), undone by ACT affine
SCORE_OFF = 504.0            # additive score offset (exact in bf16):
                             # s'/16 = 8*log2(e)*s + 31.5, the e4m3 bit value
                             # of exp(s)/8 with the Schraudolph shift; clamped
                             # at 0 on the DVE.  ACT computes exp(s - ln 8).
                             # The /8 keeps exp in e4m3 range; softmax ratios
                             # are unaffected.

def _np_bf16(a):
    return np.ascontiguousarray(a.astype(np.float32).astype(ml_dtypes.bfloat16))


def prepare_weights(Wf, bf, Wg, bg, Wh, bh, gamma):
    """Host-side weight folding. Returns dict of bf16 arrays (dram params)."""
    Wf = np.asarray(Wf, np.float32)
    Wg = np.asarray(Wg, np.float32)
    Wh = np.asarray(Wh, np.float32)
    bf = np.asarray(bf, np.float32)
    bg = np.asarray(bg, np.float32)
    bh = np.asarray(bh, np.float32)
    gamma = float(np.asarray(gamma, np.float32))

    # f-side, scaled by C1, bias as row 64; replicated across the 4 32-row
    # bands so row-group-packed score matmuls can read from any band.
    # Column 8 (paired with g-side column 8 == 1) adds SCORE_OFF to every
    # score so the DVE fp8 bit-trick can clamp at 0 instead of going
    # negative: raw scores' = C1*s + SCORE_OFF.
    # single band: with full-K score matmuls the g-side replicas must hit
    # zero rows of fS outside band 0
    wf_aug = np.zeros((128, 128), np.float32)
    wf_aug[:C, :CK] = C1 * Wf
    wf_aug[C, :CK] = C1 * bf
    wf_aug[C, CK] = SCORE_OFF

    # g-side, same replication, column 8 = ones row
    wg_aug = np.zeros((128, 128), np.float32)
    for b in range(4):
        wg_aug[:C, 32 * b: 32 * b + CK] = Wg
        wg_aug[C, 32 * b: 32 * b + CK] = bg
        wg_aug[C, 32 * b + CK] = 1.0

    # h-side scaled by 128*gamma (keeps fp8 h out of subnormals) and a
    # 128-valued denominator column at 64; the epilogue's reciprocal of
    # 128*sum(exp) then yields gamma*ctx directly: [128, 128]
    wh_aug = np.zeros((128, 128), np.float32)
    wh_aug[:C, :C] = 128.0 * gamma * Wh
    wh_aug[C, :C] = 128.0 * gamma * bh
    wh_aug[C, C] = 128.0

    return {
        "wf": _np_bf16(wf_aug),
        "wg": _np_bf16(wg_aug),
        "wh": _np_bf16(wh_aug),
    }


def _spill_excess_waits(nc, limit=1):
    """Walrus rejects HW-queue instructions carrying more than a couple of
    semaphore waits.  Move excess waits onto standalone EventSemaphore
    instructions inserted just before the offender on the same engine
    (cumulative sem-ge waits split across instructions are equivalent)."""
    n_spill = 0
    for bb in nc.main_func.blocks:
        rebuilt = []
        changed = False
        for ins in bb.instructions:
            si = ins.sync_info
            if si is not None and len(si.on_wait) > limit:
                waits = list(si.on_wait)
                for w in waits[limit:]:
                    ev = mybir.InstEventSemaphore(
                        name=f"wspill-{n_spill}", ins=[], outs=[])
                    ev.engine = ins.engine
                    ev.sync_info = mybir.SyncInfo(on_wait=[w], on_update=[])
                    rebuilt.append(ev)
                    n_spill += 1
                ins.sync_info = mybir.SyncInfo(
                    on_wait=waits[:limit], on_update=list(si.on_update))
                changed = True
            rebuilt.append(ins)
        if changed:
            bb.instructions = rebuilt
    return n_spill


def _dedup_ldweights(nc):
    """Drop an InstLdweights whose weight AP/mode is identical to the
    immediately preceding LDW on the PE queue (score-chunk and DoubleRow
    pairs reuse the same stationary operand).  Only sync-free LDWs are
    dropped so no semaphore edges are lost."""
    n_drop = 0
    for bb in nc.main_func.blocks:
        rebuilt = []
        last_key = None
        changed = False
        for ins in bb.instructions:
            tname = type(ins).__name__
            if tname == "InstLdweights":
                si = ins.sync_info
                clean = si is None or (not si.on_wait and not si.on_update)
                key = (str(ins.ins[0]), str(getattr(ins, "perf_mode", None)),
                       str(getattr(ins, "tile_position", None)),
                       str(getattr(ins, "is_transpose", None)))
                if clean and key == last_key:
                    n_drop += 1
                    changed = True
                    continue
                last_key = key
            elif tname == "InstMatmult":
                pass  # matmul leaves the stationary operand in place
            elif ins.engine == mybir.EngineType.PE:
                last_key = None
            rebuilt.append(ins)
        if changed:
            bb.instructions = rebuilt
    return n_drop


def build_bass(repeat=1, spill=True):
    """Build the per-core Bass graph (SPMD: same graph on all 8 cores).
    repeat > 1 duplicates the whole body for timing calibration."""
    nc = bass.Bass()

    x_d = nc.declare_dram_parameter("x", [P, NT * C], F32, isOutput=False)
    wf_d = nc.declare_dram_parameter("wf", [128, 128], BF16, isOutput=False)
    wg_d = nc.declare_dram_parameter("wg", [128, 128], BF16, isOutput=False)
    wh_d = nc.declare_dram_parameter("wh", [128, 128], BF16, isOutput=False)
    xt_d = nc.declare_dram_parameter("xta", [65, N], BF16, isOutput=False)
    out_d = nc.declare_dram_parameter("out", [N, C], F32, isOutput=True)

    with tile.TileContext(nc) as tc:
        for _ in range(repeat):
            _build_body(nc, tc, x_d, wf_d, wg_d, wh_d, xt_d, out_d)
    _dedup_ldweights(nc)
    if spill:
        _spill_excess_waits(nc)
    return nc


def _build_body(nc, tc, x_d, wf_d, wg_d, wh_d, xt_d, out_d):
    from contextlib import ExitStack

    with ExitStack() as ctx:
        consts = ctx.enter_context(tc.tile_pool(name="consts", bufs=1))
        sbuf = ctx.enter_context(tc.tile_pool(name="sbuf", bufs=1))
        exp_pool = ctx.enter_context(tc.tile_pool(name="expp", bufs=6))
        work = ctx.enter_context(tc.tile_pool(name="work", bufs=6))

        # ---- load x (host pre-tiled to [p, t*c] f32): contiguous DMAs ----
        x_sb = consts.tile([P, NT, C], F32)
        x3 = x_d.rearrange("p (t c) -> p t c", c=C)
        dma_engines = [nc.sync, nc.gpsimd, nc.scalar]
        for d in range(4):
            dma_engines[d % 3].dma_start(x_sb[:, ds(4 * d, 4), :],
                                         x3[:, ds(4 * d, 4), :])
        # ---- constants (small, after x on the queues) ----
        wf_sb = consts.tile([128, 128], BF16)
        wg_sb = consts.tile([128, 128], BF16)
        wh_sb = consts.tile([128, 128], BF16)
        nc.sync.dma_start(wf_sb[:], wf_d[:])
        nc.gpsimd.dma_start(wg_sb[:], wg_d[:])
        nc.scalar.dma_start(wh_sb[:], wh_d[:])


        # identity for the final-quarter PE transposes (PE is idle then)
        id_sb = consts.tile([128, 128], BF16)
        make_identity(nc, id_sb[:])

        # --- head warmup: engines are otherwise idle for ~18us of NEFF
        # startup + input DMA.  Pull the ScalarE exp table load (~2.7us)
        # and the PE HAM un-throttle (~3.4us of sustained activity) into
        # that window using zeroed scratch.
        warm = consts.tile([128, 512], BF16)
        nc.vector.memset(warm[:], 0.0)
        wtmp = consts.tile([128, 8], BF16)
        nc.scalar.activation(wtmp[:], warm[:, :8],
                             mybir.ActivationFunctionType.Exp,
                             bias=0.0, scale=1.0)
        with tc.tile_pool(name="warm_ps", bufs=1, space="PSUM") as warm_ps:
            wp = warm_ps.tile([128, 512], F32)
            for _ in range(20):
                nc.tensor.matmul(wp[:], warm[:, :128], warm[:],
                                 start=True, stop=True)

        # ACT exp bias: exp(s'/C1 + bias) = exp(s - ln 8)
        ebias = consts.tile([P, 1], F32)
        nc.vector.memset(ebias[:], float(-SCORE_OFF / C1 - np.log(8.0)))

        # ---- xT_aug [128, N] bf16: rows 0..64 host-built [x^T ; ones],
        # rows 65..127 zeroed on device ----
        xt_sb = consts.tile([128, N], BF16)
        nc.vector.memset(xt_sb[C:, :], 0.0)
        for d in range(2):
            dma_engines[d % 3].dma_start(xt_sb[:65, ds(d * HALF, HALF)],
                                         xt_d[:, ds(d * HALF, HALF)])

        with tc.tile_pool(name="pro_ps", bufs=2, space="PSUM") as pro_ps:
            # f/g projections (f scaled by C1), band-replicated
            f_sb = consts.tile([128, N], BF16)
            g_sb = consts.tile([128, N], BF16)
            for chunk in range(N // 512):
                pf = pro_ps.tile([128, 512], F32, tag="fg")
                nc.tensor.matmul(pf[:], wf_sb[:, :], xt_sb[:, ts(chunk, 512)],
                                 start=True, stop=True)
                nc.any.tensor_copy(f_sb[:, ts(chunk, 512)], pf[:])
            for chunk in range(N // 512):
                pg = pro_ps.tile([128, 512], F32, tag="fg")
                nc.tensor.matmul(pg[:], wg_sb[:, :], xt_sb[:, ts(chunk, 512)],
                                 start=True, stop=True)
                nc.any.tensor_copy(g_sb[:, ts(chunk, 512)], pg[:])

            # h_aug tiles in fp8: h_sb[:, m, :] = [128*gamma*h | 128 | 0pad];
            # adjacent m-tiles form the [128, 2, 128] DoubleRow weight pairs
            h_sb = consts.tile([P, NT, 128], FP8)
            for grp in range(NT // 4):
                ph = pro_ps.tile([128, 512], F32, tag="h")
                for j in range(4):
                    m = 4 * grp + j
                    nc.tensor.matmul(ph[:, ts(j, P)], xt_sb[:, ts(m, P)],
                                     wh_sb[:], start=True, stop=True)
                nc.any.tensor_copy(h_sb[:, ds(4 * grp, 4), :], ph[:])

        # x tiles 16..31 (quarters 2-3 residuals, needed ~90us in): behind
        # the compute-critical loads on each queue
        for d in range(4, 8):
            dma_engines[d % 3].dma_start(x_sb[:, ds(4 * d, 4), :],
                                         x3[:, ds(4 * d, 4), :])

        # ---- main: scores -> exp -> ctxT accumulate; epilogue, per n-quarter ----
        QW = 1024                      # quarter width
        NQ = N // QW                   # 4
        QT = QW // P                   # n-tiles per quarter
        with tc.tile_pool(name="ps_s", bufs=3, space="PSUM") as ps_s, \
             tc.tile_pool(name="ps_ctx", bufs=1, space="PSUM") as ps_ctx:
            # exp engine assignment: ACT chunk ~997ns vs DVE ~1192ns (+ DVE's
            # prologue/epilogue load) -> give ACT ~81 of 128 chunks, spread
            # evenly (Bresenham).
            N_CHUNKS = NQ * NT
            DVE_SHARE = 58
            use_dve = [((i * DVE_SHARE) % N_CHUNKS) < DVE_SHARE
                       for i in range(N_CHUNKS)]

            def emit_epi_head(q, ctx_ps):
                # epilogue head for quarter q: copy ctxT to SBUF bf16 (halves
                # so transposes start before the full copy) and kick off the
                # 8 DMA transposes.  Emitted DEFERRED, inside the next
                # quarter's first group, so the engine queues don't serialize
                # the quarter boundary through the PSUM->SBUF copy.
                ctxt_sb = work.tile([128, QW], BF16, tag="ctxt")
                nc.any.tensor_copy(ctxt_sb[:, :QW // 2], ctx_ps[:, :QW // 2])
                nc.any.tensor_copy(ctxt_sb[:, QW // 2:], ctx_ps[:, QW // 2:])
                o_tr = work.tile([128, QT, P], BF16, tag="otr")
                for t in range(QT):
                    (nc.sync if t % 2 == 0 else nc.scalar).dma_start_transpose(
                        o_tr[:, t, :], ctxt_sb[:, ts(t, P)])
                return o_tr

            def emit_epi_tile(q, blk, t):
                # one n-tile of quarter q's epilogue: reciprocal of the
                # denominator column, fused (ctx*rden + x), store.  Spread
                # one-per-group across the next quarter so the DVE meets
                # each DMA transpose at its natural cadence instead of
                # joining on the whole serialized chain.  The fused
                # scalar_tensor_tensor replaces a DVE mul + GpSimd add
                # (saves ~130ns DVE per tile and one cross-engine hop
                # before the store).
                rden = work.tile([P, 1], F32, tag="rden")
                nc.vector.reciprocal(rden[:], blk[:, C: C + 1])
                osb = work.tile([P, C], F32, tag="osb")
                nc.vector.scalar_tensor_tensor(
                    osb[:], blk[:, :C], rden[:],
                    x_sb[:, q * QT + t, :],
                    mybir.AluOpType.mult, mybir.AluOpType.add)
                dma_engines[(t + 1) % 3].dma_start(
                    out_d[ds((q * QT + t) * P, P), :], osb[:])

            prev_ctx = None
            prev_otr = None
            for q in range(NQ):
                ctx_ps = ps_ctx.tile([128, QW], F32, tag="ctx")
                for mg in range(NT // 2):
                    # two m-tiles' score matmuls (full-K: the zero rows of
                    # fS make the band replicas in gS harmless)
                    sp = [ps_s.tile([128, QW], F32, tag="s", name=f"s{b}")
                          for b in range(2)]
                    for b in range(2):
                        m = 2 * mg + b
                        for j in range(QW // 512):
                            nc.tensor.matmul(
                                sp[b][:, ts(j, 512)],
                                g_sb[:, ts(m, P)],
                                f_sb[:, ds(q * QW + j * 512, 512)],
                                start=True, stop=True)
                    e_pair = exp_pool.tile([128, 2, QW], FP8, tag="e")
                    for b in range(2):
                        m = 2 * mg + b
                        if use_dve[q * NT + m]:
                            _dve_exp(nc, work, e_pair, b, sp[b], QW)
                        else:
                            nc.scalar.activation(
                                e_pair[:, b, :], sp[b][:],
                                mybir.ActivationFunctionType.Exp,
                                bias=ebias[:], scale=float(1.0 / C1))
                    # previous quarter's epilogue, software-pipelined into
                    # this quarter: head at mg=0 (must precede this quarter's
                    # first ctx matmuls -- ps_ctx has bufs=1, so the copies
                    # must be emitted before the bank reuse), one tile's tail
                    # per group at mg=2..9
                    if prev_ctx is not None:
                        if mg == 0:
                            prev_otr = emit_epi_head(q - 1, prev_ctx)
                        if 2 <= mg <= 9:
                            emit_epi_tile(q - 1, prev_otr[:, mg - 2, :],
                                          mg - 2)
                    for j in range(QW // 512):
                        nc.tensor.matmul(
                            ctx_ps[:, ts(j, 512)],
                            h_sb[:, ds(2 * mg, 2), :],
                            e_pair[:, :, ts(j, 512)],
                            perf_mode=mybir.MatmulPerfMode.DoubleRow,
                            start=(mg == 0), stop=(mg == NT // 2 - 1))
                prev_ctx = ctx_ps

            # tail: last quarter's epilogue.  Nothing left for the PE, so
            # transpose on it instead of the DMA xbar; the exp engines are
            # done and run the per-tile tails back-to-back.
            ctxt_sb = work.tile([128, QW], BF16, tag="ctxt")
            nc.any.tensor_copy(ctxt_sb[:, :QW // 2], prev_ctx[:, :QW // 2])
            nc.any.tensor_copy(ctxt_sb[:, QW // 2:], prev_ctx[:, QW // 2:])
            tr_ps = ps_s.tile([128, QW], BF16, tag="s", name="trps")
            for t in range(QT):
                nc.tensor.transpose(tr_ps[:, ts(t, P)],
                                    ctxt_sb[:, ts(t, P)], id_sb[:])
            for t in range(QT):
                emit_epi_tile(NQ - 1, tr_ps[:, ts(t, P)], t)


def _dve_exp(nc, work, e_pair, b, s_ps, ncols):
    """fp8e4m3 bit-trick exp on the DVE: i8 = round(max(s'/16, 0))
    reinterpreted as e4m3 ~= exp(s)/8.  s' = C1*s + SCORE_OFF (from the
    weights), so s'/16 = 8*log2(e)*s + 31.5 -- the e4m3 bit pattern of
    exp(s)/8 with the Schraudolph shift; ultra-negative scores clamp to +0."""
    i8_view = e_pair.bitcast(mybir.dt.int8)
    nc.vector.tensor_scalar(i8_view[:, b, :ncols], s_ps[:, :ncols],
                            1.0 / 16.0, 0.0,
                            mybir.AluOpType.mult, mybir.AluOpType.max)


_CACHE = {}


def _get_nc():
    if "nc" not in _CACHE:
        _CACHE["nc"] = build_bass()
    return _CACHE["nc"]


def kernel(x, Wf, bf, Wg, bg, Wh, bh, gamma):
    x = np.asarray(x, np.float32)
    B = x.shape[0]
    assert x.shape == (B, 64, 64, 64) and B == 8

    w = prepare_weights(Wf, bf, Wg, bg, Wh, bh, gamma)
    nc = _get_nc()
    xt = x.reshape(B, NT, P, C).transpose(0, 2, 1, 3).reshape(B, P, NT * C)
    xta = np.ones((B, 65, N), np.float32)
    xta[:, :C, :] = x.reshape(B, N, C).transpose(0, 2, 1)
    xta = xta.astype(ml_dtypes.bfloat16)
    in_maps = [{"x": np.ascontiguousarray(xt[i]),
                "xta": np.ascontiguousarray(xta[i]), **w} for i in range(B)]
    res = run_bass_kernel_spmd(nc, in_maps, core_ids=list(range(8)))
    out = np.stack([np.asarray(res.results[i]["out"]).reshape(64, 64, 64)
                    for i in range(B)])
    return out.astype(np.float32)
